# revision 22
# baseline (speedup 1.0000x reference)
"""BilinearInteraction (field_interaction) on 8 TRN2 NeuronCores.

  out[b,f,d] = emb[b,f,d] * sum_{g!=f, e} W[f,g,d,e] * emb[b,g,e]

Strategy (data-parallel, per sharding hint):
  - Host: fold the f!=g mask into W and permute it to a single GEMM matrix
    w2[g*32+e, f*32+d]; flatten embeddings to x[B, 1280]; shard batch over
    8 cores (2048 rows each); replicate w2. Ship x pre-TRANSPOSED and
    pre-packed into the exact SBUF layout [p=128, kt=10, b=2048] so every
    device DMA is a matching-AP slice copy.
  - Device (per core) computes the FLIPPED product
        sT = w2.T @ xT   (i.e. s = x @ w2, stored transposed)
    with w2 k-tiles as the STATIONARY operand — w2's natural [k, n] layout
    is exactly the lhsT the PE wants — and xT k-tiles as the MOVING
    operand. The epilogue outT = xT * sT reads the same xT tiles (n and k
    share the (field,dim) flattening), so xT ships once and no transpose
    ever touches the PE.
  - Precision: fp16 in (PSUM accumulates fp32), fp16 out (host upcasts);
    rel err ~8e-4 vs the 2e-2 gate. fp16 moving rate is 1 col/cycle ->
    204800 PE cycles = 85.3us is the per-core floor; fp8 would halve it
    but measures 3.8e-2 rel err — fails the gate.
  - dma_start issue costs ~0.61us SERIALIZED on the SP sequencer (after a
    fixed ~7.2us framework preamble), so DMA COUNT is the ramp currency:
    10 w k-tiles + 10 x bc0-quarters interleaved feed the first wave
    (8 output units kt-interleaved across all 8 PSUM banks, so the PE
    starts ~10.5us in and consumes pairs as they land), then the rest of
    xT ships as just TWO big strided DMAs (bc1, bc23). Outputs leave as
    one DMA per n-tile (the last n-tile split in two to shorten the
    post-last-matmul tail). 4 PE-warmup matmuls on the first-landed x
    quarter burn the DVFS p-state ramp inside the DMA wait.
"""

from contextlib import ExitStack

import numpy as np

BATCH = 16384
NUM_FIELDS = 40
EMBED_DIM = 32
N_CORES = 8

B_LOCAL = BATCH // N_CORES   # 2048
K = NUM_FIELDS * EMBED_DIM   # 1280 (contraction dim == output dim N)
P = 128
NKT = K // P                 # 10 k-tiles
NNT = K // P                 # 10 n-tiles
BC = 512                     # moving free-dim per matmul (PSUM bank = 512 fp32)
NBC = B_LOCAL // BC          # 4 b-chunks
# PE warmup: bridge the PE from its ~7.3-8.0us preamble exit to first-data
# at ~10.8-11.0us with NO idle gap (a >=0.7us gap measurably resets the
# DVFS ramp and costs ~2us of MID-clock matmuls). The HAM clock ramps on
# array ACTIVITY (~3.4us of heavy matmuls), so the bulk of the bridge is
# heavy 512-col warmups (427ns each at MID clock), finished with light
# 128-col ones (~110ns) so overshoot only delays the stream marginally.
N_WARM_HEAVY = 7
N_WARM_LIGHT = 3
FB = K + BC                  # first-block tensor: w2 k-tile 0 ++ x (0,bc0)
WAVE = 8                     # first-wave units == PSUM banks

_NC_CACHE = {}


def _build_kernel():
    import concourse.bacc as bacc
    import concourse.mybir as mybir
    import concourse.tile as tile

    F32 = mybir.dt.float32
    FP16 = mybir.dt.float16

    nc = bacc.Bacc("TRN2", target_bir_lowering=False, debug=False, num_devices=N_CORES)

    xt_d = nc.declare_dram_parameter("xt", [P, NBC, NKT, BC], FP16, isOutput=False)
    w_d = nc.declare_dram_parameter("w2", [K, K], FP16, isOutput=False)
    fb_d = nc.declare_dram_parameter("fb", [P, FB], FP16, isOutput=False)
    o_d = nc.declare_dram_parameter("out", [NNT, P, B_LOCAL], FP16, isOutput=True)

    with tile.TileContext(nc) as tc, ExitStack() as ctx:
        wpool = ctx.enter_context(tc.tile_pool(name="w", bufs=1))
        xpool = ctx.enter_context(tc.tile_pool(name="x", bufs=1))
        opool = ctx.enter_context(tc.tile_pool(name="o", bufs=1))
        accps = ctx.enter_context(tc.tile_pool(name="acc", bufs=8, space="PSUM"))

        # bc-major x layout: the big bc1/bc23 loads write per-partition
        # CONTIGUOUS ranges (kt-strided 1KB-chunk writes measurably slow
        # concurrent matmuls ~21% via SBUF write contention)
        x_all = xpool.tile([P, NBC, NKT, BC], FP16, name="x_all", tag="x")
        fb_sb = wpool.tile([P, FB], FP16, name="fb_sb", tag="fb")
        w_sb = [None] + [wpool.tile([P, K], FP16, name=f"w{kt}", tag=f"w{kt}")
                         for kt in range(1, NKT)]
        o_sb = [opool.tile([P, B_LOCAL], FP16, name=f"o{nt}", tag=f"o{nt}")
                for nt in range(NNT)]
        wsrc = xpool.tile([P, BC], FP16, name="wsrc", tag="wsrc")

        # --- PE warmup FIRST, on a DVE-memset tile (contents don't
        # matter, the PSUM slot is reset by the first real accumulation):
        # no DMA dependency, so the PE starts right after its ~7.2us
        # preamble and the DVFS clock is fully ramped (~3us of busy) by
        # the time the first real inputs land.
        nc.vector.memset(wsrc[:], 0.0)
        warm = accps.tile([P, BC], F32, name="warm", tag="acc")
        for _ in range(N_WARM_HEAVY):
            nc.tensor.matmul(warm[:], wsrc[:, 0:P], wsrc[:],
                             start=True, stop=True)
        for _ in range(N_WARM_LIGHT):
            nc.tensor.matmul(warm[:, 0:P], wsrc[:, 0:P], wsrc[:, 0:P],
                             start=True, stop=True)

        # --- Input DMAs, in issue order == need order (one logical queue
        # drains FIFO): the fused first block (w k-tile 0 ++ x quarter
        # (0,bc0)) as a single issue+transfer chain, then per-kt
        # (x bc0 quarter, w k-tile) pairs for the rest of the first wave,
        # then two big strided loads for the rest of x.
        # fb goes out on the Scalar engine's HWDGE ring: its sequencer
        # reaches its first DMA slot ~1us before SP's, and fb's completion
        # doesn't queue behind the pair-DMA packets on the SP ring.
        nc.scalar.dma_start(fb_sb[:], fb_d[:])
        for kt in range(1, NKT):
            nc.sync.dma_start(x_all[:, 0, kt, :], xt_d[:, 0, kt, :])
            nc.sync.dma_start(w_sb[kt][:], w_d[kt * P:(kt + 1) * P, :])
        nc.sync.dma_start(x_all[:, 1, :, :], xt_d[:, 1, :, :])
        nc.sync.dma_start(x_all[:, 2:4, :, :], xt_d[:, 2:4, :, :])

        accs = {}

        def w_ap(kt, lo, hi):
            return fb_sb[:, lo:hi] if kt == 0 else w_sb[kt][:, lo:hi]

        def x_ap(kt, bc):
            if kt == 0 and bc == 0:
                return fb_sb[:, K:K + BC]
            return x_all[:, bc, kt, :]

        def emit_unit_mm(nt, bc, kt):
            nc.tensor.matmul(accs[(nt, bc)][:],
                             w_ap(kt, nt * P, (nt + 1) * P),
                             x_ap(kt, bc),
                             start=(kt == 0), stop=(kt == NKT - 1))

        def emit_epilogue(nt, bc):
            nc.vector.tensor_mul(o_sb[nt][:, bc * BC:(bc + 1) * BC],
                                 x_ap(nt, bc),
                                 accs[(nt, bc)][:])
            del accs[(nt, bc)]

        # --- First wave: units (nt 0..7, bc 0), kt-interleaved across all
        # 8 PSUM banks so each (w[kt], x[kt]q0) pair is consumed the moment
        # it lands.
        for nt in range(WAVE):
            accs[(nt, 0)] = accps.tile([P, BC], F32, name=f"acc{nt}_0",
                                       tag="acc")
        for kt in range(NKT):
            for nt in range(WAVE):
                emit_unit_mm(nt, 0, kt)
        for nt in range(WAVE):
            emit_epilogue(nt, 0)

        # --- Steady state: one unit at a time, back-to-back matmuls.
        # bc-outer order matches x arrival (bc1 lands ~24us, bc2/3 ~31us).
        rest = [(nt, 0) for nt in range(WAVE, NNT)]
        rest += [(nt, bc) for bc in range(1, NBC) for nt in range(NNT)]
        for nt, bc in rest:
            accs[(nt, bc)] = accps.tile([P, BC], F32, name=f"acc{nt}_{bc}",
                                        tag="acc")
            for kt in range(NKT):
                emit_unit_mm(nt, bc, kt)
            emit_epilogue(nt, bc)
            # Output DMAs: one per n-tile once its 4th quarter is done
            # (bc-outer sweep => that's in the bc=3 sweep). The final
            # n-tile ships its first three quarters early (after bc=2)
            # so only a small 128KB DMA trails the very last matmul.
            if nt == NNT - 1 and bc == NBC - 2:
                nc.sync.dma_start(o_d[nt][:, 0:3 * BC],
                                  o_sb[nt][:, 0:3 * BC])
            if bc == NBC - 1:
                if nt < NNT - 1:
                    nc.sync.dma_start(o_d[nt][:], o_sb[nt][:])
                else:
                    nc.sync.dma_start(o_d[nt][:, 3 * BC:4 * BC],
                                      o_sb[nt][:, 3 * BC:4 * BC])

    nc.compile()
    return nc


def _get_nc():
    if "nc" not in _NC_CACHE:
        _NC_CACHE["nc"] = _build_kernel()
    return _NC_CACHE["nc"]


def _ensure_axon_hooks():
    """bass_utils imports antenv.axon_hooks when BASS_TRACE is set; provide a
    no-op registry if the environment lacks it so tracing degrades gracefully
    instead of crashing."""
    try:
        import antenv.axon_hooks  # noqa: F401
    except ImportError:
        import sys
        import types

        m = types.ModuleType("antenv.axon_hooks")
        m._HOOK = None
        m.set_axon_ntff_profile_hook = lambda h: setattr(m, "_HOOK", h)
        m.get_axon_ntff_profile_hook = lambda: m._HOOK
        sys.modules["antenv.axon_hooks"] = m


def _prepare_in_maps(embeddings: np.ndarray, bilinear_W: np.ndarray):
    """Host-side prep: mask+permute W to the GEMM matrix, transpose x
    shards into the device SBUF layout [p, kt, b], fp16 everything."""
    embeddings = np.ascontiguousarray(np.asarray(embeddings, dtype=np.float32))
    bilinear_W = np.ascontiguousarray(np.asarray(bilinear_W, dtype=np.float32))
    F, D = NUM_FIELDS, EMBED_DIM

    # w2[g*D+e, f*D+d] = W[f,g,d,e] * (f != g)
    mask = (1.0 - np.eye(F, dtype=np.float32))[:, :, None, None]
    w2 = np.ascontiguousarray(
        (bilinear_W * mask).transpose(1, 3, 0, 2).reshape(F * D, F * D)
    ).astype(np.float16)

    x = embeddings.reshape(BATCH, F * D).astype(np.float16)
    in_maps = []
    for c in range(N_CORES):
        shard = x[c * B_LOCAL:(c + 1) * B_LOCAL]          # [2048, 1280]
        # xt[p, bc, kt, c] = shard[b = bc*512 + c, kt*128 + p]
        xt = np.ascontiguousarray(
            shard.reshape(NBC, BC, NKT, P).transpose(3, 0, 2, 1))
        # fused first block: w2 k-tile 0 ++ x quarter (kt=0, bc=0)
        fb = np.ascontiguousarray(
            np.concatenate([w2[0:P, :], xt[:, 0, 0, :]], axis=1))
        in_maps.append({"xt": xt, "w2": w2, "fb": fb})
    return in_maps


def _finish_output(results) -> np.ndarray:
    """Un-transpose + upcast the fp16 outT shards (out[nt, p, b])."""
    shards = []
    for i in range(N_CORES):
        ot = np.asarray(results[i]["out"])               # [10, 128, 2048]
        shards.append(ot.transpose(2, 0, 1).reshape(B_LOCAL, K))
    out = np.concatenate(shards, axis=0).astype(np.float32)
    return out.reshape(BATCH, NUM_FIELDS, EMBED_DIM)


def kernel(embeddings: np.ndarray, bilinear_W: np.ndarray) -> np.ndarray:
    _ensure_axon_hooks()
    from concourse.bass_utils import run_bass_kernel_spmd

    in_maps = _prepare_in_maps(embeddings, bilinear_W)
    nc = _get_nc()
    res = run_bass_kernel_spmd(nc, in_maps, list(range(N_CORES)))
    return _finish_output(res.results)
